# revision 40
# baseline (speedup 1.0000x reference)
"""Trainium2 Bass kernel for nn_MultiHeadAttention (B=2, S=2048, D=1024, H=16).

Sharding: 8 cores = 2 batch groups x 4 cores. Core c handles batch c//4 and
heads 4*(c%4) .. 4*(c%4)+4 (two head-pairs p=0,1). Each core computes Q/K/V
projections for its batch+heads, transposed-layout attention, and a partial
output projection over its 256 head-dims. Host sums the 4 partials per batch.

v2 design (vs the f32r baseline):
- fp16 inputs/weights (halves HBM traffic + SBUF), softmax scale folded into Wq.
- exp runs on ACT directly PSUM->SBUF as fp16 pt tiles (no separate evict).
- attn@V matmuls are column-tiled pairs (head A -> PE cols 0:64, head B ->
  cols 64:128) running concurrently at full array width (M=64 each, one
  [128,512] PSUM tile holds both heads' context).
- softmax denominators: DVE pairwise-tree over the 16 fp16 pt tiles, then a
  ones[128,2] matmul for the 128-key partition reduction, reciprocal on DVE,
  and a selector-matmul broadcast; one DVE multiply normalizes both heads
  (head B lands on partitions 64:128 -> no DMA broadcast round-trip).
- out-projection partials are DMA'd directly PSUM->HBM.
- K-projection+Q0 ramp interleaved with the first attention block's scores;
  V/Q projections and out-projection woven into PE slack of the ACT-paced
  attention sweep.
"""

import numpy as np

B, S, D, H = 2, 2048, 1024, 16
HD = D // H          # 64
NCORES = 8
HPC = 4              # heads per core
CHD = HPC * HD       # 256 head-dims per core
TOK = S              # tokens per core (one batch)
QW = 512             # query window
NQW = TOK // QW      # 4 windows
NKT = TOK // 128     # 16 key tiles
SCALE = 1.0 / np.sqrt(np.float32(D))  # 1/32, folded into Wq on host

_PROG = None  # cached compiled program
_LAST_IN_MAPS = None  # stashed per-core inputs (for external profiling)


def _build():
    from contextlib import ExitStack

    import concourse.bass as bass
    import concourse.tile as tile
    from concourse import bacc, mybir

    F16 = mybir.dt.float16
    F32 = mybir.dt.float32
    EXP = mybir.ActivationFunctionType.Exp

    nc = bacc.Bacc("TRN2", target_bir_lowering=False, debug=False,
                   num_devices=NCORES)

    xqT = nc.dram_tensor("xqT", [D, TOK], F16, kind="ExternalInput").ap()
    xkT = nc.dram_tensor("xkT", [D, TOK], F16, kind="ExternalInput").ap()
    xvT = nc.dram_tensor("xvT", [D, TOK], F16, kind="ExternalInput").ap()
    wqT = nc.dram_tensor("wqT", [D, CHD], F16, kind="ExternalInput").ap()
    wkT = nc.dram_tensor("wkT", [D, CHD], F16, kind="ExternalInput").ap()
    wvT = nc.dram_tensor("wvT", [D, CHD], F16, kind="ExternalInput").ap()
    woTs = nc.dram_tensor("woTs", [CHD, D], F16, kind="ExternalInput").ap()
    pout = nc.dram_tensor("pout", [TOK, D], F16, kind="ExternalOutput").ap()

    CB = 512            # projection column-block width (tokens)
    NCB = TOK // CB     # 4 blocks

    with tile.TileContext(nc) as tc, ExitStack() as ctx:
        const = ctx.enter_context(tc.tile_pool(name="const", bufs=1))
        wq_sb = const.tile([128, 8, CHD], F16, tag="wq")
        wk_sb = const.tile([128, 8, CHD], F16, tag="wk")
        wv_sb = const.tile([128, 8, CHD], F16, tag="wv")
        wo_sb = [const.tile([128, D], F16, tag=f"wo{p}", name=f"wo{p}")
                 for p in range(2)]
        # ones for the denominator partition-sum matmul
        onesK = const.tile([128, 1], F16, tag="onesK")
        # ones row for the reciprocal broadcast matmuls
        ones1 = const.tile([1, 128], F16, tag="ones1")

        # only wq/wk are needed in the ramp; wv/wo are DMA'd later so the
        # critical xq0/xk loads aren't queued behind them
        nc.sync.dma_start(out=wq_sb, in_=wqT.rearrange("(ko ki) m -> ki ko m", ki=128))
        nc.sync.dma_start(out=wk_sb, in_=wkT.rearrange("(ko ki) m -> ki ko m", ki=128))

        nc.vector.memset(onesK, 1.0)
        nc.vector.memset(ones1, 1.0)

        # warm the exp table early
        warm = const.tile([1, 8], F32, tag="warm")
        nc.vector.memset(warm, 0.0)
        nc.scalar.activation(out=warm, in_=warm, func=EXP)

        big = ctx.enter_context(tc.tile_pool(name="big", bufs=1))
        KT = big.tile([128, 2, TOK], F16, tag="kt")          # [hd, pair, keys]
        vnat = big.tile([128, NKT, CHD], F16, tag="vnat")    # [keys, kt, hd]
        ctxP = [big.tile([128, TOK], F16, tag=f"ctxP{p}", name=f"ctxP{p}")
                for p in range(2)]

        xkb = ctx.enter_context(tc.tile_pool(name="xkb", bufs=3))
        xkb2 = ctx.enter_context(tc.tile_pool(name="xkb2", bufs=2))
        xvb = ctx.enter_context(tc.tile_pool(name="xvb", bufs=3))
        xqb = ctx.enter_context(tc.tile_pool(name="xqb", bufs=2))
        qtp = ctx.enter_context(tc.tile_pool(name="qtp", bufs=2))
        ptp = ctx.enter_context(tc.tile_pool(name="ptp", bufs=2))
        dtp = ctx.enter_context(tc.tile_pool(name="dtp", bufs=2))
        rrp = ctx.enter_context(tc.tile_pool(name="rrp", bufs=2))
        oev = ctx.enter_context(tc.tile_pool(name="oev", bufs=3))

        # PSUM (8 banks): sc ring 2x[128,1024] (4) + cop ring 2x[128,512] (2)
        # + shared proj/out/den/bcast pool 2x[128,512] (2)
        scp = ctx.enter_context(tc.tile_pool(name="scp", bufs=2, space="PSUM"))
        copp = ctx.enter_context(tc.tile_pool(name="copp", bufs=2, space="PSUM"))
        pp = ctx.enter_context(tc.tile_pool(name="pp", bufs=2, space="PSUM"))

        # ---------- emission helpers ----------
        def dma_x_block(pool, src, c0, w, name, queue=None):
            t = pool.tile([128, 8, w], F16, tag="xb", name=name)
            (queue or nc.sync).dma_start(
                out=t,
                in_=src[:, c0:c0 + w].rearrange("(ko ki) t -> ki ko t", ki=128))
            return t

        def proj_q(qw_i, xq_t):
            """Project one query window -> qt [128, 2, QW] fp16."""
            qt = qtp.tile([128, 2, QW], F16, tag="qt", name=f"qt{qw_i}")
            for m in range(2):
                psq = pp.tile([128, QW], F32, tag="pp", name=f"psq{qw_i}_{m}")
                for ko in range(8):
                    nc.tensor.matmul(
                        psq[:], wq_sb[:, ko, m * 128:(m + 1) * 128],
                        xq_t[:, ko, :], start=(ko == 0), stop=(ko == 7))
                nc.vector.tensor_copy(qt[:, m, :], psq[:])
            return qt

        def proj_k_block(c0, w, xk_t):
            # one w-token column block, per head-pair m
            for m in range(2):
                psk = pp.tile([128, w], F32, tag="pp", name=f"psk{c0}_{m}")
                for ko in range(8):
                    nc.tensor.matmul(
                        psk[:], wk_sb[:, ko, m * 128:(m + 1) * 128],
                        xk_t[:, ko, :], start=(ko == 0), stop=(ko == 7))
                nc.vector.tensor_copy(KT[:, m, c0:c0 + w], psk[:])

        def proj_v_block(blk, xv_t):
            # out = x_blockT.T @ Wv -> natural layout; 512 tokens = kt 4b..4b+3
            for half in range(2):
                pv = pp.tile([128, 2 * CHD], F32, tag="pp",
                             name=f"pv{blk}_{half}")
                for tt in range(2):
                    t0 = (2 * half + tt) * 128
                    for ko in range(8):
                        nc.tensor.matmul(
                            pv[:, tt * CHD:(tt + 1) * CHD],
                            xv_t[:, ko, t0:t0 + 128],
                            wv_sb[:, ko, :], start=(ko == 0), stop=(ko == 7))
                nc.vector.tensor_copy(
                    vnat[:, 4 * blk + 2 * half:4 * blk + 2 * half + 2, :],
                    bass.AP(tensor=pv.tensor, offset=pv.offset,
                            ap=[list(pv.ap[0]), [CHD, 2], [1, CHD]]))

        def emit_scores(p, kt_i, qt, name):
            k0 = kt_i * 128
            sc = scp.tile([128, 2 * QW], F32, tag="sc", name=name)
            nc.tensor.matmul(
                sc[:, 0:QW], KT[0:64, p, k0:k0 + 128], qt[0:64, p, :],
                start=True, stop=True, tile_position=(0, 0))
            nc.tensor.matmul(
                sc[:, QW:2 * QW], KT[64:128, p, k0:k0 + 128], qt[64:128, p, :],
                start=True, stop=True, tile_position=(64, 0))
            return sc

        def emit_exp(sc, ptblk, kt_i):
            nc.scalar.activation(out=ptblk[:, kt_i, :], in_=sc[:], func=EXP)

        def emit_ctx(p, kt_i, ptblk, cop):
            h0 = p * 128
            nc.tensor.matmul(
                cop[0:64, :], vnat[:, kt_i, h0:h0 + 64],
                ptblk[:, kt_i, 0:QW],
                start=(kt_i == 0), stop=(kt_i == NKT - 1),
                tile_position=(0, 0))
            nc.tensor.matmul(
                cop[64:128, :], vnat[:, kt_i, h0 + 64:h0 + 128],
                ptblk[:, kt_i, QW:2 * QW],
                start=(kt_i == 0), stop=(kt_i == NKT - 1),
                tile_position=(0, 64))

        def emit_den_tree_lo(ptblk, blk_i):
            """First half of the DVE tree: sum kt 0..7 -> slots 4:6."""
            st = dtp.tile([128, 2, 2 * QW], F16, tag="dt", name=f"dt{blk_i}")
            pv = lambda a, b: ptblk[:, a:b, :]
            nc.vector.tensor_add(st[:, 0:2, :], pv(0, 2), pv(2, 4))     # A
            nc.vector.tensor_add(pv(0, 2), pv(4, 6), pv(6, 8))          # B
            nc.vector.tensor_add(pv(4, 6), st[:, 0:2, :], pv(0, 2))     # E=A+B
            return st

        def emit_den_tree_hi(ptblk, st, eng=None):
            """Second half: sum kt 8..15, combine -> acc [128, 1024].
            C/D optionally run on the otherwise-idle gpsimd engine."""
            e = eng or nc.vector
            pv = lambda a, b: ptblk[:, a:b, :]
            e.tensor_add(pv(2, 4), pv(8, 10), pv(10, 12))               # C
            e.tensor_add(pv(6, 8), pv(12, 14), pv(14, 16))              # D
            nc.vector.tensor_add(pv(8, 10), pv(2, 4), pv(6, 8))         # F=C+D
            nc.vector.tensor_add(pv(0, 2), pv(4, 6), pv(8, 10))         # G=E+F
            nc.vector.tensor_add(ptblk[:, 2, :], ptblk[:, 0, :],
                                 ptblk[:, 1, :])                        # acc
            return ptblk[:, 2, :]

        def emit_den_tree(ptblk, blk_i):
            st = emit_den_tree_lo(ptblk, blk_i)
            return emit_den_tree_hi(ptblk, st, eng=nc.gpsimd)

        def emit_den_norm(p, qw_i, accv, cop, blk_i):
            """Partition-reduce via ones-matmul, reciprocal, ones-broadcast
            matmuls, normalize both heads into ctxP."""
            # den_h [1, 512] = ones.T @ acc_h (everything on partition 0:
            # engines can't address a partition range at an unaligned base)
            rrc = rrp.tile([1, 2 * QW], F32, tag="rrc", name=f"rrc{blk_i}")
            for h in range(2):
                den = pp.tile([128, QW], F32, tag="pp", name=f"den{blk_i}_{h}")
                nc.tensor.matmul(
                    den[0:1, :], onesK[:, 0:1], accv[:, h * QW:(h + 1) * QW],
                    start=True, stop=True)
                nc.vector.reciprocal_approx_fast(
                    rrc[0:1, h * QW:(h + 1) * QW], den[0:1, :])
            # fp32 matmuls run two passes -> cast rrc to fp16 on idle gpsimd
            rrc16 = rrp.tile([1, 2 * QW], F16, tag="rrc16",
                             name=f"rrc16_{blk_i}")
            nc.gpsimd.tensor_copy(rrc16[:], rrc[:])
            # broadcast via two ones-matmuls: head A -> parts 0:64, B -> 64:128
            bc = pp.tile([128, QW], F32, tag="pp", name=f"bc{blk_i}")
            nc.tensor.matmul(bc[0:64, :], ones1[0:1, 0:64], rrc16[0:1, 0:QW],
                             start=True, stop=True, tile_position=(0, 0))
            nc.tensor.matmul(bc[64:128, :], ones1[0:1, 0:64],
                             rrc16[0:1, QW:2 * QW],
                             start=True, stop=True, tile_position=(0, 64))
            # both tensor_tensor operands can't be PSUM -> stage bc in SBUF
            bcs = rrp.tile([128, QW], F32, tag="bcs", name=f"bcs{blk_i}")
            nc.vector.tensor_copy(bcs[:], bc[:])
            # normalize both heads at once into ctxP (fp16)
            nc.vector.tensor_mul(
                ctxP[p][:, qw_i * QW:(qw_i + 1) * QW], cop[:], bcs[:])

        def emit_outproj_one(qw_i, c):
            # c in 0..7 enumerates (tt, et)
            tt, et = c // 2, c % 2
            t0 = qw_i * QW + tt * 128
            po = pp.tile([128, 512], F32, tag="pp",
                         name=f"po{qw_i}_{tt}_{et}")
            for p in range(2):
                nc.tensor.matmul(
                    po[:], ctxP[p][:, t0:t0 + 128],
                    wo_sb[p][:, et * 512:(et + 1) * 512],
                    start=(p == 0), stop=(p == 1))
            # gpsimd has no PSUM port -> evict on DVE (fp16), DMA from SBUF
            ev = oev.tile([128, 512], F16, tag="oev")
            nc.vector.tensor_copy(ev[:], po[:])
            nc.sync.dma_start(
                out=pout[t0:t0 + 128, et * 512:(et + 1) * 512],
                in_=ev[:])

        def emit_outproj(qw_i):
            for c in range(8):
                emit_outproj_one(qw_i, c)

        # ---------- program ----------
        # single hwdge queue, need-ordered; the first K blocks are narrow so
        # the first score chunks aren't gated on big serialized transfers
        KW = [256, 256, 512, 512, 512]
        KC0 = [0, 256, 512, 1024, 1536]
        xq_t = [None] * NQW
        xq_t[0] = dma_x_block(xqb, xqT, 0, QW, "xq0")
        xk_t = [dma_x_block(xkb if w == 512 else xkb2, xkT, c0, w, f"xk{c0}")
                for c0, w in zip(KC0, KW)]
        xv_t = [dma_x_block(xvb, xvT, b * CB, CB, f"xv{b}") for b in range(2)]
        nc.sync.dma_start(out=wv_sb, in_=wvT.rearrange("(ko ki) m -> ki ko m", ki=128))

        qt0 = proj_q(0, xq_t[0])

        # ramp: K projection feeds scores+exp for BOTH p-blocks of window 0,
        # so ACT saturates as early as possible.
        ptblk0 = ptp.tile([128, NKT, 2 * QW], F16, tag="pt", name="pt0")
        ptblk1 = ptp.tile([128, NKT, 2 * QW], F16, tag="pt", name="pt1")
        ptb = [ptblk0, ptblk1]

        def ramp_chunk(p, kt_i):
            sc = emit_scores(p, kt_i, qt0, f"sc_r{p}_{kt_i}")
            emit_exp(sc, ptb[p], kt_i)

        for b in range(len(KW)):
            proj_k_block(KC0[b], KW[b], xk_t[b])
            for kt_i in range(KC0[b] // 128, (KC0[b] + KW[b]) // 128):
                ramp_chunk(0, kt_i)
        xv_t.append(dma_x_block(xvb, xvT, 2 * CB, CB, "xv2"))
        xq_t[1] = dma_x_block(xqb, xqT, QW, QW, "xq1")
        xv_t.append(dma_x_block(xvb, xvT, 3 * CB, CB, "xv3"))
        nc.sync.dma_start(out=wo_sb[0], in_=woTs[0:128, :])
        nc.sync.dma_start(out=wo_sb[1], in_=woTs[128:256, :])
        for kt_i in range(NKT):
            ramp_chunk(1, kt_i)

        # V projection + block-0 ctx (trails the ACT backlog); block-0 den
        cop0 = copp.tile([128, QW], F32, tag="cop", name="cop0")
        cop1 = copp.tile([128, QW], F32, tag="cop", name="cop1")
        for b in range(NCB):
            proj_v_block(b, xv_t[b])
            for kt_i in range(4 * b, 4 * b + 4):
                emit_ctx(0, kt_i, ptblk0, cop0)
        # qt1 FIRST: its DVE eviction gates block 2's scores; the block-0
        # denominator chain has a whole block of slack
        qt1 = proj_q(1, xq_t[1])
        acc0 = emit_den_tree(ptblk0, 0)
        emit_den_norm(0, 0, acc0, cop0, 0)

        qt_cur = qt1
        qt_next = None
        # pending from the previous block: ctx batches drained one per jj
        # (4 batches for block 1 whose ctx hasn't started, 1 for others)
        pend_ctx = (1, ptblk1, cop1, [0, 1, 2, 3])
        pend_norm = (1, 0, ptblk1, cop1)
        pend_late = True   # pend block's exps finish only during this block

        blocks = [(qw_i, p) for qw_i in range(NQW) for p in range(2)][2:]
        for bi, (qw_i, p) in enumerate(blocks):
            blk_i = bi + 2
            last = bi == len(blocks) - 1
            ptblk = ptp.tile([128, NKT, 2 * QW], F16, tag="pt",
                             name=f"pt{blk_i}")
            cop = copp.tile([128, QW], F32, tag="cop", name=f"cop{blk_i}")
            qt_b = qt_cur
            for jj in range(4):
                # 4 score+exp chunks (one mode), then 4 lagged ctx pairs
                for kt_i in range(4 * jj, 4 * jj + 4):
                    sc = emit_scores(p, kt_i, qt_b, f"sc{blk_i}_{kt_i}")
                    emit_exp(sc, ptblk, kt_i)
                if pend_ctx is not None and pend_ctx[3]:
                    q = pend_ctx[3].pop(0)
                    for kt_i in range(4 * q, 4 * q + 4):
                        emit_ctx(pend_ctx[0], kt_i, pend_ctx[1], pend_ctx[2])
                if jj >= 1:
                    for kt_i in range(4 * jj - 4, 4 * jj):
                        emit_ctx(p, kt_i, ptblk, cop)
                if jj == 0 and p == 1 and qw_i + 1 < NQW:
                    # next window's queries FIRST: their DVE evictions gate
                    # the next block's scores; den work has a block of slack
                    qt_next = proj_q(qw_i + 1, xq_t[qw_i + 1])
                if pend_norm is not None:
                    # previous block's denominator: tree then matmuls/norm
                    if jj == (2 if pend_late else 1):
                        pend_acc = emit_den_tree(pend_norm[2], blk_i - 1)
                    if jj == (3 if pend_late else 2):
                        emit_den_norm(pend_norm[0], pend_norm[1], pend_acc,
                                      pend_norm[3], blk_i - 1)
                        pend_norm = None
                if jj >= 2 and p == 1 and qw_i >= 1:
                    # previous window's out-projection, spread over 2 batches
                    for c in range(4 * (jj - 2), 4 * (jj - 2) + 4):
                        emit_outproj_one(qw_i - 1, c)
                if jj == 2 and p == 0 and qw_i + 1 < NQW:
                    xq_t[qw_i + 1] = dma_x_block(
                        xqb, xqT, (qw_i + 1) * QW, QW, f"xq{qw_i + 1}")
                if jj == 3 and last:
                    # head start on the final block's denominator: kt 0..7
                    # plus the kt 8..11 partial (C) -> only D/F/G/acc remain
                    st_last = emit_den_tree_lo(ptblk, blk_i)
                    nc.vector.tensor_add(
                        ptblk[:, 2:4, :], ptblk[:, 8:10, :],
                        ptblk[:, 10:12, :])
            pend_ctx = (p, ptblk, cop, [3])
            pend_norm = (p, qw_i, ptblk, cop)
            pend_late = False
            if p == 1 and qw_i + 1 < NQW:
                qt_cur = qt_next

        # tail: last block's final ctx batch, den hi-remainder, then a
        # token-tile-pipelined norm + out-projection to shorten the chain
        p_l, ptblk_l, cop_l = pend_ctx[0], pend_ctx[1], pend_ctx[2]
        for kt_i in range(NKT - 4, NKT):
            emit_ctx(p_l, kt_i, ptblk_l, cop_l)
        pv = lambda a, b: ptblk_l[:, a:b, :]
        nc.vector.tensor_add(pv(6, 8), pv(12, 14), pv(14, 16))      # D
        nc.vector.tensor_add(pv(8, 10), pv(2, 4), pv(6, 8))         # F=C+D
        nc.vector.tensor_add(pv(0, 2), pv(4, 6), pv(8, 10))         # G
        nc.vector.tensor_add(ptblk_l[:, 2, :], ptblk_l[:, 0, :],
                             ptblk_l[:, 1, :])
        accv = ptblk_l[:, 2, :]
        qw_l = pend_norm[1]
        rrc = rrp.tile([1, 2 * QW], F32, tag="rrc", name="rrc8")
        rrc16 = rrp.tile([1, 2 * QW], F16, tag="rrc16", name="rrc16_8")
        for h in range(2):
            den = pp.tile([128, QW], F32, tag="pp", name=f"den8_{h}")
            nc.tensor.matmul(den[0:1, :], onesK[:, 0:1],
                             accv[:, h * QW:(h + 1) * QW],
                             start=True, stop=True)
            nc.vector.reciprocal_approx_fast(
                rrc[0:1, h * QW:(h + 1) * QW], den[0:1, :])
            # per-head cast so the broadcast matmul isn't gated on both heads
            nc.gpsimd.tensor_copy(rrc16[0:1, h * QW:(h + 1) * QW],
                                  rrc[0:1, h * QW:(h + 1) * QW])
        bc = pp.tile([128, QW], F32, tag="pp", name="bc8")
        nc.tensor.matmul(bc[0:64, :], ones1[0:1, 0:64], rrc16[0:1, 0:QW],
                         start=True, stop=True, tile_position=(0, 0))
        nc.tensor.matmul(bc[64:128, :], ones1[0:1, 0:64],
                         rrc16[0:1, QW:2 * QW],
                         start=True, stop=True, tile_position=(0, 64))
        bcs = rrp.tile([128, QW], F32, tag="bcs", name="bcs8")
        nc.vector.tensor_copy(bcs[:], bc[:])
        # per-token-tile: normalize 128 columns, then immediately emit the
        # out-projection chunks that need only those columns
        for tt in range(4):
            c0, c1 = tt * 128, (tt + 1) * 128
            nc.vector.tensor_mul(
                ctxP[p_l][:, qw_l * QW + c0:qw_l * QW + c1],
                cop_l[:, c0:c1], bcs[:, c0:c1])
            emit_outproj_one(NQW - 1, 2 * tt)
            emit_outproj_one(NQW - 1, 2 * tt + 1)

    nc.compile()
    return nc


def kernel(query, key, value, Wq, Wk, Wv, Wo):
    global _PROG, _LAST_IN_MAPS
    from concourse.bass_utils import run_bass_kernel_spmd

    if _PROG is None:
        _PROG = _build()
    nc = _PROG

    q2 = np.asarray(query, dtype=np.float32).reshape(B, S, D)
    k2 = np.asarray(key, dtype=np.float32).reshape(B, S, D)
    v2 = np.asarray(value, dtype=np.float32).reshape(B, S, D)
    Wq = np.asarray(Wq, dtype=np.float32)
    Wk = np.asarray(Wk, dtype=np.float32)
    Wv = np.asarray(Wv, dtype=np.float32)
    Wo = np.asarray(Wo, dtype=np.float32)

    xT = {}
    for b in range(B):
        xT[("q", b)] = np.ascontiguousarray(q2[b].T).astype(np.float16)
        xT[("k", b)] = np.ascontiguousarray(k2[b].T).astype(np.float16)
        xT[("v", b)] = np.ascontiguousarray(v2[b].T).astype(np.float16)

    in_maps = []
    for c in range(NCORES):
        b = c // 4
        l = c % 4
        rs = slice(CHD * l, CHD * (l + 1))
        in_maps.append({
            "xqT": xT[("q", b)],
            "xkT": xT[("k", b)],
            "xvT": xT[("v", b)],
            "wqT": (Wq[rs, :].T * SCALE).astype(np.float16),
            "wkT": Wk[rs, :].T.astype(np.float16),
            "wvT": Wv[rs, :].T.astype(np.float16),
            "woTs": np.ascontiguousarray(Wo[:, rs].T).astype(np.float16),
        })

    _LAST_IN_MAPS = in_maps
    res = run_bass_kernel_spmd(nc, in_maps, core_ids=list(range(NCORES)))
    parts = [res.results[c]["pout"].astype(np.float32) for c in range(NCORES)]
    out = np.empty((B, S, D), dtype=np.float32)
    for b in range(B):
        out[b] = parts[4 * b] + parts[4 * b + 1] + parts[4 * b + 2] + parts[4 * b + 3]
    return out


# revision 41
# speedup vs baseline: 1.0596x; 1.0596x over previous
"""Trainium2 Bass kernel for nn_MultiHeadAttention (B=2, S=2048, D=1024, H=16).

Sharding: 8 cores = 2 batch groups x 4 cores. Core c handles batch c//4 and
heads 4*(c%4) .. 4*(c%4)+4 (two head-pairs p=0,1). Each core computes Q/K/V
projections for its batch+heads, transposed-layout attention, and a partial
output projection over its 256 head-dims. Host sums the 4 partials per batch.

v2 design (vs the f32r baseline):
- fp16 inputs/weights (halves HBM traffic + SBUF), softmax scale folded into Wq.
- exp runs on ACT directly PSUM->SBUF as fp16 pt tiles (no separate evict).
- attn@V matmuls are column-tiled pairs (head A -> PE cols 0:64, head B ->
  cols 64:128) running concurrently at full array width (M=64 each, one
  [128,512] PSUM tile holds both heads' context).
- softmax denominators: DVE pairwise-tree over the 16 fp16 pt tiles, then a
  ones[128,2] matmul for the 128-key partition reduction, reciprocal on DVE,
  and a selector-matmul broadcast; one DVE multiply normalizes both heads
  (head B lands on partitions 64:128 -> no DMA broadcast round-trip).
- out-projection partials are DMA'd directly PSUM->HBM.
- K-projection+Q0 ramp interleaved with the first attention block's scores;
  V/Q projections and out-projection woven into PE slack of the ACT-paced
  attention sweep.
"""

import numpy as np

B, S, D, H = 2, 2048, 1024, 16
HD = D // H          # 64
NCORES = 8
HPC = 4              # heads per core
CHD = HPC * HD       # 256 head-dims per core
TOK = S              # tokens per core (one batch)
QW = 512             # query window
NQW = TOK // QW      # 4 windows
NKT = TOK // 128     # 16 key tiles
SCALE = 1.0 / np.sqrt(np.float32(D))  # 1/32, folded into Wq on host

_PROG = None  # cached compiled program
_LAST_IN_MAPS = None  # stashed per-core inputs (for external profiling)


def _build():
    from contextlib import ExitStack

    import concourse.bass as bass
    import concourse.tile as tile
    from concourse import bacc, mybir

    F16 = mybir.dt.float16
    F32 = mybir.dt.float32
    EXP = mybir.ActivationFunctionType.Exp

    nc = bacc.Bacc("TRN2", target_bir_lowering=False, debug=False,
                   num_devices=NCORES)

    xqT = nc.dram_tensor("xqT", [D, TOK], F16, kind="ExternalInput").ap()
    xkT = nc.dram_tensor("xkT", [D, TOK], F16, kind="ExternalInput").ap()
    xvT = nc.dram_tensor("xvT", [D, TOK], F16, kind="ExternalInput").ap()
    wqT = nc.dram_tensor("wqT", [D, CHD], F16, kind="ExternalInput").ap()
    wkT = nc.dram_tensor("wkT", [D, CHD], F16, kind="ExternalInput").ap()
    wvT = nc.dram_tensor("wvT", [D, CHD], F16, kind="ExternalInput").ap()
    woTs = nc.dram_tensor("woTs", [CHD, D], F16, kind="ExternalInput").ap()
    pout = nc.dram_tensor("pout", [TOK, D], F16, kind="ExternalOutput").ap()

    CB = 512            # projection column-block width (tokens)
    NCB = TOK // CB     # 4 blocks

    with tile.TileContext(nc) as tc, ExitStack() as ctx:
        const = ctx.enter_context(tc.tile_pool(name="const", bufs=1))
        wq_sb = const.tile([128, 8, CHD], F16, tag="wq")
        wk_sb = const.tile([128, 8, CHD], F16, tag="wk")
        wv_sb = const.tile([128, 8, CHD], F16, tag="wv")
        wo_sb = [const.tile([128, D], F16, tag=f"wo{p}", name=f"wo{p}")
                 for p in range(2)]
        # ones for the denominator partition-sum matmul
        onesK = const.tile([128, 1], F16, tag="onesK")
        # ones row for the reciprocal broadcast matmuls
        ones1 = const.tile([1, 128], F16, tag="ones1")

        # only wq/wk are needed in the ramp; wv/wo are DMA'd later so the
        # critical xq0/xk loads aren't queued behind them
        nc.sync.dma_start(out=wq_sb, in_=wqT.rearrange("(ko ki) m -> ki ko m", ki=128))
        nc.sync.dma_start(out=wk_sb, in_=wkT.rearrange("(ko ki) m -> ki ko m", ki=128))

        nc.vector.memset(onesK, 1.0)
        nc.vector.memset(ones1, 1.0)

        # warm the exp table early
        warm = const.tile([1, 8], F32, tag="warm")
        nc.vector.memset(warm, 0.0)
        nc.scalar.activation(out=warm, in_=warm, func=EXP)

        big = ctx.enter_context(tc.tile_pool(name="big", bufs=1))
        KT = big.tile([128, 2, TOK], F16, tag="kt")          # [hd, pair, keys]
        vnat = big.tile([128, NKT, CHD], F16, tag="vnat")    # [keys, kt, hd]
        ctxP = [big.tile([128, TOK], F16, tag=f"ctxP{p}", name=f"ctxP{p}")
                for p in range(2)]

        xkb = ctx.enter_context(tc.tile_pool(name="xkb", bufs=3))
        xkb2 = ctx.enter_context(tc.tile_pool(name="xkb2", bufs=2))
        xvb = ctx.enter_context(tc.tile_pool(name="xvb", bufs=3))
        xqb = ctx.enter_context(tc.tile_pool(name="xqb", bufs=2))
        qtp = ctx.enter_context(tc.tile_pool(name="qtp", bufs=2))
        ptp = ctx.enter_context(tc.tile_pool(name="ptp", bufs=2))
        dtp = ctx.enter_context(tc.tile_pool(name="dtp", bufs=2))
        rrp = ctx.enter_context(tc.tile_pool(name="rrp", bufs=2))
        oev = ctx.enter_context(tc.tile_pool(name="oev", bufs=3))

        # PSUM (8 banks): sc ring 2x[128,1024] (4) + cop ring 2x[128,512] (2)
        # + shared proj/out/den/bcast pool 2x[128,512] (2)
        scp = ctx.enter_context(tc.tile_pool(name="scp", bufs=2, space="PSUM"))
        copp = ctx.enter_context(tc.tile_pool(name="copp", bufs=2, space="PSUM"))
        pp = ctx.enter_context(tc.tile_pool(name="pp", bufs=2, space="PSUM"))

        # ---------- emission helpers ----------
        def dma_x_block(pool, src, c0, w, name, queue=None):
            t = pool.tile([128, 8, w], F16, tag="xb", name=name)
            (queue or nc.sync).dma_start(
                out=t,
                in_=src[:, c0:c0 + w].rearrange("(ko ki) t -> ki ko t", ki=128))
            return t

        def proj_q(qw_i, xq_t):
            """Project one query window -> qt [128, 2, QW] fp16."""
            qt = qtp.tile([128, 2, QW], F16, tag="qt", name=f"qt{qw_i}")
            for m in range(2):
                psq = pp.tile([128, QW], F32, tag="pp", name=f"psq{qw_i}_{m}")
                for ko in range(8):
                    nc.tensor.matmul(
                        psq[:], wq_sb[:, ko, m * 128:(m + 1) * 128],
                        xq_t[:, ko, :], start=(ko == 0), stop=(ko == 7))
                nc.vector.tensor_copy(qt[:, m, :], psq[:])
            return qt

        def proj_k_block(c0, w, xk_t):
            # one w-token column block, per head-pair m
            for m in range(2):
                psk = pp.tile([128, w], F32, tag="pp", name=f"psk{c0}_{m}")
                for ko in range(8):
                    nc.tensor.matmul(
                        psk[:], wk_sb[:, ko, m * 128:(m + 1) * 128],
                        xk_t[:, ko, :], start=(ko == 0), stop=(ko == 7))
                nc.vector.tensor_copy(KT[:, m, c0:c0 + w], psk[:])

        def proj_v_block(blk, xv_t):
            # out = x_blockT.T @ Wv -> natural layout; 512 tokens = kt 4b..4b+3
            for half in range(2):
                pv = pp.tile([128, 2 * CHD], F32, tag="pp",
                             name=f"pv{blk}_{half}")
                for tt in range(2):
                    t0 = (2 * half + tt) * 128
                    for ko in range(8):
                        nc.tensor.matmul(
                            pv[:, tt * CHD:(tt + 1) * CHD],
                            xv_t[:, ko, t0:t0 + 128],
                            wv_sb[:, ko, :], start=(ko == 0), stop=(ko == 7))
                nc.vector.tensor_copy(
                    vnat[:, 4 * blk + 2 * half:4 * blk + 2 * half + 2, :],
                    bass.AP(tensor=pv.tensor, offset=pv.offset,
                            ap=[list(pv.ap[0]), [CHD, 2], [1, CHD]]))

        def emit_scores(p, kt_i, qt, name):
            k0 = kt_i * 128
            sc = scp.tile([128, 2 * QW], F32, tag="sc", name=name)
            nc.tensor.matmul(
                sc[:, 0:QW], KT[0:64, p, k0:k0 + 128], qt[0:64, p, :],
                start=True, stop=True, tile_position=(0, 0))
            nc.tensor.matmul(
                sc[:, QW:2 * QW], KT[64:128, p, k0:k0 + 128], qt[64:128, p, :],
                start=True, stop=True, tile_position=(64, 0))
            return sc

        def emit_exp(sc, ptblk, kt_i):
            nc.scalar.activation(out=ptblk[:, kt_i, :], in_=sc[:], func=EXP)

        def emit_ctx(p, kt_i, ptblk, cop):
            h0 = p * 128
            nc.tensor.matmul(
                cop[0:64, :], vnat[:, kt_i, h0:h0 + 64],
                ptblk[:, kt_i, 0:QW],
                start=(kt_i == 0), stop=(kt_i == NKT - 1),
                tile_position=(0, 0))
            nc.tensor.matmul(
                cop[64:128, :], vnat[:, kt_i, h0 + 64:h0 + 128],
                ptblk[:, kt_i, QW:2 * QW],
                start=(kt_i == 0), stop=(kt_i == NKT - 1),
                tile_position=(0, 64))

        def emit_den_tree_lo(ptblk, blk_i):
            """First half of the DVE tree: sum kt 0..7 -> slots 4:6."""
            st = dtp.tile([128, 2, 2 * QW], F16, tag="dt", name=f"dt{blk_i}")
            pv = lambda a, b: ptblk[:, a:b, :]
            nc.vector.tensor_add(st[:, 0:2, :], pv(0, 2), pv(2, 4))     # A
            nc.vector.tensor_add(pv(0, 2), pv(4, 6), pv(6, 8))          # B
            nc.vector.tensor_add(pv(4, 6), st[:, 0:2, :], pv(0, 2))     # E=A+B
            return st

        def emit_den_tree_hi(ptblk, st, eng=None):
            """Second half: sum kt 8..15, combine -> acc [128, 1024].
            C/D optionally run on the otherwise-idle gpsimd engine."""
            e = eng or nc.vector
            pv = lambda a, b: ptblk[:, a:b, :]
            e.tensor_add(pv(2, 4), pv(8, 10), pv(10, 12))               # C
            e.tensor_add(pv(6, 8), pv(12, 14), pv(14, 16))              # D
            nc.vector.tensor_add(pv(8, 10), pv(2, 4), pv(6, 8))         # F=C+D
            nc.vector.tensor_add(pv(0, 2), pv(4, 6), pv(8, 10))         # G=E+F
            nc.vector.tensor_add(ptblk[:, 2, :], ptblk[:, 0, :],
                                 ptblk[:, 1, :])                        # acc
            return ptblk[:, 2, :]

        def emit_den_tree(ptblk, blk_i):
            st = emit_den_tree_lo(ptblk, blk_i)
            return emit_den_tree_hi(ptblk, st)

        def emit_den_norm(p, qw_i, accv, cop, blk_i):
            """Partition-reduce via ones-matmul, reciprocal, ones-broadcast
            matmuls, normalize both heads into ctxP."""
            # den_h [1, 512] = ones.T @ acc_h (everything on partition 0:
            # engines can't address a partition range at an unaligned base)
            rrc = rrp.tile([1, 2 * QW], F32, tag="rrc", name=f"rrc{blk_i}")
            for h in range(2):
                den = pp.tile([128, QW], F32, tag="pp", name=f"den{blk_i}_{h}")
                nc.tensor.matmul(
                    den[0:1, :], onesK[:, 0:1], accv[:, h * QW:(h + 1) * QW],
                    start=True, stop=True)
                nc.vector.reciprocal_approx_fast(
                    rrc[0:1, h * QW:(h + 1) * QW], den[0:1, :])
            # fp32 matmuls run two passes -> cast rrc to fp16 on idle gpsimd
            rrc16 = rrp.tile([1, 2 * QW], F16, tag="rrc16",
                             name=f"rrc16_{blk_i}")
            nc.gpsimd.tensor_copy(rrc16[:], rrc[:])
            # broadcast via two ones-matmuls: head A -> parts 0:64, B -> 64:128
            bc = pp.tile([128, QW], F32, tag="pp", name=f"bc{blk_i}")
            nc.tensor.matmul(bc[0:64, :], ones1[0:1, 0:64], rrc16[0:1, 0:QW],
                             start=True, stop=True, tile_position=(0, 0))
            nc.tensor.matmul(bc[64:128, :], ones1[0:1, 0:64],
                             rrc16[0:1, QW:2 * QW],
                             start=True, stop=True, tile_position=(0, 64))
            # both tensor_tensor operands can't be PSUM -> stage bc in SBUF
            bcs = rrp.tile([128, QW], F32, tag="bcs", name=f"bcs{blk_i}")
            nc.vector.tensor_copy(bcs[:], bc[:])
            # normalize both heads at once into ctxP (fp16)
            nc.vector.tensor_mul(
                ctxP[p][:, qw_i * QW:(qw_i + 1) * QW], cop[:], bcs[:])

        def emit_outproj_one(qw_i, c):
            # c in 0..7 enumerates (tt, et)
            tt, et = c // 2, c % 2
            t0 = qw_i * QW + tt * 128
            po = pp.tile([128, 512], F32, tag="pp",
                         name=f"po{qw_i}_{tt}_{et}")
            for p in range(2):
                nc.tensor.matmul(
                    po[:], ctxP[p][:, t0:t0 + 128],
                    wo_sb[p][:, et * 512:(et + 1) * 512],
                    start=(p == 0), stop=(p == 1))
            # gpsimd has no PSUM port -> evict on DVE (fp16), DMA from SBUF
            ev = oev.tile([128, 512], F16, tag="oev")
            nc.vector.tensor_copy(ev[:], po[:])
            nc.sync.dma_start(
                out=pout[t0:t0 + 128, et * 512:(et + 1) * 512],
                in_=ev[:])

        def emit_outproj(qw_i):
            for c in range(8):
                emit_outproj_one(qw_i, c)

        # ---------- program ----------
        # single hwdge queue, need-ordered; the first K blocks are narrow so
        # the first score chunks aren't gated on big serialized transfers
        KW = [256, 256, 512, 512, 512]
        KC0 = [0, 256, 512, 1024, 1536]
        xq_t = [None] * NQW
        xq_t[0] = dma_x_block(xqb, xqT, 0, QW, "xq0")
        xk_t = [dma_x_block(xkb if w == 512 else xkb2, xkT, c0, w, f"xk{c0}")
                for c0, w in zip(KC0, KW)]
        xv_t = [dma_x_block(xvb, xvT, b * CB, CB, f"xv{b}") for b in range(2)]
        nc.sync.dma_start(out=wv_sb, in_=wvT.rearrange("(ko ki) m -> ki ko m", ki=128))

        qt0 = proj_q(0, xq_t[0])

        # ramp: K projection feeds scores+exp for BOTH p-blocks of window 0,
        # so ACT saturates as early as possible.
        ptblk0 = ptp.tile([128, NKT, 2 * QW], F16, tag="pt", name="pt0")
        ptblk1 = ptp.tile([128, NKT, 2 * QW], F16, tag="pt", name="pt1")
        ptb = [ptblk0, ptblk1]

        def ramp_chunk(p, kt_i):
            sc = emit_scores(p, kt_i, qt0, f"sc_r{p}_{kt_i}")
            emit_exp(sc, ptb[p], kt_i)

        for b in range(len(KW)):
            proj_k_block(KC0[b], KW[b], xk_t[b])
            for kt_i in range(KC0[b] // 128, (KC0[b] + KW[b]) // 128):
                ramp_chunk(0, kt_i)
        xv_t.append(dma_x_block(xvb, xvT, 2 * CB, CB, "xv2"))
        xq_t[1] = dma_x_block(xqb, xqT, QW, QW, "xq1")
        xv_t.append(dma_x_block(xvb, xvT, 3 * CB, CB, "xv3"))
        nc.sync.dma_start(out=wo_sb[0], in_=woTs[0:128, :])
        nc.sync.dma_start(out=wo_sb[1], in_=woTs[128:256, :])
        for kt_i in range(NKT):
            ramp_chunk(1, kt_i)

        # V projection + block-0 ctx (trails the ACT backlog); block-0 den
        cop0 = copp.tile([128, QW], F32, tag="cop", name="cop0")
        cop1 = copp.tile([128, QW], F32, tag="cop", name="cop1")
        for b in range(NCB):
            proj_v_block(b, xv_t[b])
            for kt_i in range(4 * b, 4 * b + 4):
                emit_ctx(0, kt_i, ptblk0, cop0)
        # qt1 FIRST: its DVE eviction gates block 2's scores; the block-0
        # denominator chain has a whole block of slack
        qt1 = proj_q(1, xq_t[1])
        acc0 = emit_den_tree(ptblk0, 0)
        emit_den_norm(0, 0, acc0, cop0, 0)

        qt_cur = qt1
        qt_next = None
        # pending from the previous block: ctx batches drained one per jj
        # (4 batches for block 1 whose ctx hasn't started, 1 for others)
        pend_ctx = (1, ptblk1, cop1, [0, 1, 2, 3])
        pend_norm = (1, 0, ptblk1, cop1)
        pend_late = True   # pend block's exps finish only during this block

        blocks = [(qw_i, p) for qw_i in range(NQW) for p in range(2)][2:]
        for bi, (qw_i, p) in enumerate(blocks):
            blk_i = bi + 2
            last = bi == len(blocks) - 1
            ptblk = ptp.tile([128, NKT, 2 * QW], F16, tag="pt",
                             name=f"pt{blk_i}")
            cop = copp.tile([128, QW], F32, tag="cop", name=f"cop{blk_i}")
            qt_b = qt_cur
            for jj in range(4):
                # 4 score+exp chunks (one mode), then 4 lagged ctx pairs
                for kt_i in range(4 * jj, 4 * jj + 4):
                    sc = emit_scores(p, kt_i, qt_b, f"sc{blk_i}_{kt_i}")
                    emit_exp(sc, ptblk, kt_i)
                if pend_ctx is not None and pend_ctx[3]:
                    q = pend_ctx[3].pop(0)
                    for kt_i in range(4 * q, 4 * q + 4):
                        emit_ctx(pend_ctx[0], kt_i, pend_ctx[1], pend_ctx[2])
                if jj >= 1:
                    for kt_i in range(4 * jj - 4, 4 * jj):
                        emit_ctx(p, kt_i, ptblk, cop)
                if jj == 0 and p == 1 and qw_i + 1 < NQW:
                    # next window's queries FIRST: their DVE evictions gate
                    # the next block's scores; den work has a block of slack
                    qt_next = proj_q(qw_i + 1, xq_t[qw_i + 1])
                if pend_norm is not None:
                    # previous block's denominator: tree then matmuls/norm
                    if jj == (2 if pend_late else 1):
                        pend_acc = emit_den_tree(pend_norm[2], blk_i - 1)
                    if jj == (3 if pend_late else 2):
                        emit_den_norm(pend_norm[0], pend_norm[1], pend_acc,
                                      pend_norm[3], blk_i - 1)
                        pend_norm = None
                if jj >= 2 and p == 1 and qw_i >= 1:
                    # previous window's out-projection, spread over 2 batches
                    for c in range(4 * (jj - 2), 4 * (jj - 2) + 4):
                        emit_outproj_one(qw_i - 1, c)
                if jj == 2 and p == 0 and qw_i + 1 < NQW:
                    xq_t[qw_i + 1] = dma_x_block(
                        xqb, xqT, (qw_i + 1) * QW, QW, f"xq{qw_i + 1}")
                if jj == 3 and last:
                    # head start on the final block's denominator: kt 0..7
                    # plus the kt 8..11 partial (C) -> only D/F/G/acc remain
                    st_last = emit_den_tree_lo(ptblk, blk_i)
                    nc.vector.tensor_add(
                        ptblk[:, 2:4, :], ptblk[:, 8:10, :],
                        ptblk[:, 10:12, :])
            pend_ctx = (p, ptblk, cop, [3])
            pend_norm = (p, qw_i, ptblk, cop)
            pend_late = False
            if p == 1 and qw_i + 1 < NQW:
                qt_cur = qt_next

        # tail: last block's final ctx batch, den hi-remainder, then a
        # token-tile-pipelined norm + out-projection to shorten the chain
        p_l, ptblk_l, cop_l = pend_ctx[0], pend_ctx[1], pend_ctx[2]
        for kt_i in range(NKT - 4, NKT):
            emit_ctx(p_l, kt_i, ptblk_l, cop_l)
        pv = lambda a, b: ptblk_l[:, a:b, :]
        nc.vector.tensor_add(pv(6, 8), pv(12, 14), pv(14, 16))      # D
        nc.vector.tensor_add(pv(8, 10), pv(2, 4), pv(6, 8))         # F=C+D
        nc.vector.tensor_add(pv(0, 2), pv(4, 6), pv(8, 10))         # G
        nc.vector.tensor_add(ptblk_l[:, 2, :], ptblk_l[:, 0, :],
                             ptblk_l[:, 1, :])
        accv = ptblk_l[:, 2, :]
        qw_l = pend_norm[1]
        rrc = rrp.tile([1, 2 * QW], F32, tag="rrc", name="rrc8")
        rrc16 = rrp.tile([1, 2 * QW], F16, tag="rrc16", name="rrc16_8")
        for h in range(2):
            den = pp.tile([128, QW], F32, tag="pp", name=f"den8_{h}")
            nc.tensor.matmul(den[0:1, :], onesK[:, 0:1],
                             accv[:, h * QW:(h + 1) * QW],
                             start=True, stop=True)
            nc.vector.reciprocal_approx_fast(
                rrc[0:1, h * QW:(h + 1) * QW], den[0:1, :])
            # per-head cast so the broadcast matmul isn't gated on both heads
            nc.gpsimd.tensor_copy(rrc16[0:1, h * QW:(h + 1) * QW],
                                  rrc[0:1, h * QW:(h + 1) * QW])
        bc = pp.tile([128, QW], F32, tag="pp", name="bc8")
        nc.tensor.matmul(bc[0:64, :], ones1[0:1, 0:64], rrc16[0:1, 0:QW],
                         start=True, stop=True, tile_position=(0, 0))
        nc.tensor.matmul(bc[64:128, :], ones1[0:1, 0:64],
                         rrc16[0:1, QW:2 * QW],
                         start=True, stop=True, tile_position=(0, 64))
        bcs = rrp.tile([128, QW], F32, tag="bcs", name="bcs8")
        nc.vector.tensor_copy(bcs[:], bc[:])
        # per-token-tile: normalize 128 columns, then immediately emit the
        # out-projection chunks that need only those columns
        for tt in range(4):
            c0, c1 = tt * 128, (tt + 1) * 128
            nc.vector.tensor_mul(
                ctxP[p_l][:, qw_l * QW + c0:qw_l * QW + c1],
                cop_l[:, c0:c1], bcs[:, c0:c1])
            emit_outproj_one(NQW - 1, 2 * tt)
            emit_outproj_one(NQW - 1, 2 * tt + 1)

    nc.compile()
    return nc


def kernel(query, key, value, Wq, Wk, Wv, Wo):
    global _PROG, _LAST_IN_MAPS
    from concourse.bass_utils import run_bass_kernel_spmd

    if _PROG is None:
        _PROG = _build()
    nc = _PROG

    q2 = np.asarray(query, dtype=np.float32).reshape(B, S, D)
    k2 = np.asarray(key, dtype=np.float32).reshape(B, S, D)
    v2 = np.asarray(value, dtype=np.float32).reshape(B, S, D)
    Wq = np.asarray(Wq, dtype=np.float32)
    Wk = np.asarray(Wk, dtype=np.float32)
    Wv = np.asarray(Wv, dtype=np.float32)
    Wo = np.asarray(Wo, dtype=np.float32)

    xT = {}
    for b in range(B):
        xT[("q", b)] = np.ascontiguousarray(q2[b].T).astype(np.float16)
        xT[("k", b)] = np.ascontiguousarray(k2[b].T).astype(np.float16)
        xT[("v", b)] = np.ascontiguousarray(v2[b].T).astype(np.float16)

    in_maps = []
    for c in range(NCORES):
        b = c // 4
        l = c % 4
        rs = slice(CHD * l, CHD * (l + 1))
        in_maps.append({
            "xqT": xT[("q", b)],
            "xkT": xT[("k", b)],
            "xvT": xT[("v", b)],
            "wqT": (Wq[rs, :].T * SCALE).astype(np.float16),
            "wkT": Wk[rs, :].T.astype(np.float16),
            "wvT": Wv[rs, :].T.astype(np.float16),
            "woTs": np.ascontiguousarray(Wo[:, rs].T).astype(np.float16),
        })

    _LAST_IN_MAPS = in_maps
    res = run_bass_kernel_spmd(nc, in_maps, core_ids=list(range(NCORES)))
    parts = [res.results[c]["pout"].astype(np.float32) for c in range(NCORES)]
    out = np.empty((B, S, D), dtype=np.float32)
    for b in range(B):
        out[b] = parts[4 * b] + parts[4 * b + 1] + parts[4 * b + 2] + parts[4 * b + 3]
    return out
